# revision 6
# baseline (speedup 1.0000x reference)
"""Conv2d-via-FFT reference implemented as a direct convolution on TRN2.

The reference pads to FFT size 61 >= 32+3-1, so its circular cross-correlation
equals the linear valid cross-correlation: out[n,f,i,j] =
sum_{c,p,q} x[n,c,i+p,j+q] * w[f,c,p,q] + bias[f].  That is an ordinary
stride-1 valid conv2d, which maps directly onto the PE array as 9 accumulated
matmuls (one per filter tap) with C=128 on the contraction partitions.

Sharding: data-parallel over N (64 samples -> 8 per core), filter replicated.
"""

import numpy as np

import concourse.bass as bass
import concourse.tile as tile
import concourse.bacc as bacc
import concourse.mybir as mybir
from concourse.bass_utils import run_bass_kernel_spmd
from concourse.vector_clock import ScopedClock


class FastEpilogueTileContext(tile.TileContext):
    """TileContext whose exit skips the two all-engine EVSEM butterfly
    barriers (~8us).  The drains still carry the full global-clock wait set,
    and the gpsimd sem clear is ordered behind its own drained engine, so
    re-execution of the NEFF starts from zeroed semaphores as before."""

    def _drain_and_barrier(self, tick_clock, wait_clock):
        nc = self.nc
        clock = ScopedClock({None: tick_clock.global_clock})
        for eng in (nc.sync, nc.scalar, nc.gpsimd):
            d = eng.drain()
            wait_clock.add_sem_waits(d.ins, clock)
        popped = nc._tile_sem_poison_stack.pop()
        assert popped is self._sem_poison
        nc.clear_and_free_semaphores(list(self.sems.allocated().values()))

dt = mybir.dt
F32 = dt.float32
F32R = dt.float32r

N, C, H, W = 64, 128, 32, 32
F, KH, KW = 128, 3, 3
OH, OW = H - KH + 1, W - KW + 1          # 30, 30
NCORES = 8
NPC = N // NCORES                        # samples per core
ROWS_PER_CHUNK = 15                      # 2 chunks of 15 rows -> 450 px per matmul
NCHUNK = OH // ROWS_PER_CHUNK
CHUNK_PX = ROWS_PER_CHUNK * OW           # 450 <= 512 (one PSUM bank)


def _build():
    nc = bacc.Bacc("TRN2", target_bir_lowering=False, debug=False)

    # x and w are declared float32r: raw fp32 bits are fed straight to the
    # PE's reduced-precision fp32 path via fast HWDGE DMAs (no cast DMA).
    x_d = nc.dram_tensor("x", [C, NPC, H, W], F32R, kind="ExternalInput").ap()
    w_d = nc.dram_tensor("w", [C, KH * KW, F], F32R, kind="ExternalInput").ap()
    b_d = nc.dram_tensor("bias", [F, 1], F32, kind="ExternalInput").ap()
    o_d = nc.dram_tensor("out", [NPC, F, OH * OW], F32, kind="ExternalOutput").ap()

    with FastEpilogueTileContext(nc) as tc:
        with tc.tile_pool(name="const", bufs=1) as const_pool, \
             tc.tile_pool(name="xp", bufs=3) as xp, \
             tc.tile_pool(name="ps", bufs=4, space="PSUM") as ps, \
             tc.tile_pool(name="ob", bufs=4) as ob:
            # w is loaded tap-by-tap (9 x 64KB) on the Scalar HWDGE queue so
            # the first matmul only waits for tap 0; x streams per-sample in
            # row halves on the Sync queue so chunk-0 can start after rows
            # 0..16 land.
            w_sb = const_pool.tile([C, KH * KW, F], F32R)
            b_sb = const_pool.tile([F, 1], F32)
            nc.scalar.dma_start(w_sb[:, 0], w_d[:, 0])
            x_tiles = []
            for n in range(NPC):
                x_sb = xp.tile([C, H, W], F32R, tag="x", name=f"x_sb{n}")
                nc.sync.dma_start(x_sb[:, 0:17], x_d[:, n, 0:17])
                nc.sync.dma_start(x_sb[:, 17:32], x_d[:, n, 17:32])
                x_tiles.append(x_sb)
            nc.scalar.dma_start(b_sb[:], b_d[:])
            for k in range(1, KH * KW):
                nc.scalar.dma_start(w_sb[:, k], w_d[:, k])

            for n in range(NPC):
                x_sb = x_tiles[n]
                for r in range(NCHUNK):
                    acc = ps.tile([F, CHUNK_PX], F32)
                    for k in range(KH * KW):
                        p, q = divmod(k, KW)
                        r0 = r * ROWS_PER_CHUNK + p
                        nc.tensor.matmul(
                            acc[:],
                            w_sb[:, k],
                            x_sb[:, r0:r0 + ROWS_PER_CHUNK, q:q + OW],
                            start=(k == 0),
                            stop=(k == KH * KW - 1),
                        )
                    o_sb = ob.tile([F, CHUNK_PX], F32)
                    nc.scalar.activation(
                        o_sb[:], acc[:],
                        mybir.ActivationFunctionType.Identity,
                        bias=b_sb[:],
                    )
                    nc.scalar.dma_start(
                        o_d[n, :, r * CHUNK_PX:(r + 1) * CHUNK_PX], o_sb[:],
                    )

    nc.compile()
    return nc


_NC = None


def _get_nc():
    global _NC
    if _NC is None:
        _NC = _build()
    return _NC


def _in_maps(x, w, bias):
    w_prep = np.ascontiguousarray(
        w.transpose(1, 2, 3, 0).reshape(C, KH * KW, F).astype(np.float32))
    b_prep = np.ascontiguousarray(bias.astype(np.float32).reshape(F, 1))
    maps = []
    for c in range(NCORES):
        xc = np.ascontiguousarray(
            x[c * NPC:(c + 1) * NPC].transpose(1, 0, 2, 3).astype(np.float32))
        maps.append({"x": xc, "w": w_prep, "bias": b_prep})
    return maps


def run(x, w, bias, trace=False, **spmd_kwargs):
    """Run the SPMD kernel; returns (out [N,F,OH,OW], BassKernelResults)."""
    nc = _get_nc()
    res = run_bass_kernel_spmd(nc, _in_maps(x, w, bias), list(range(NCORES)),
                               trace=trace, **spmd_kwargs)
    parts = [res.results[c]["out"].reshape(NPC, F, OH, OW) for c in range(NCORES)]
    return np.concatenate(parts, axis=0), res


def kernel(x, w, bias):
    out, _ = run(np.asarray(x), np.asarray(w), np.asarray(bias))
    return out


# revision 10
# speedup vs baseline: 1.0089x; 1.0089x over previous
"""Conv2d-via-FFT reference implemented as a direct convolution on TRN2.

The reference pads to FFT size 61 >= 32+3-1, so its circular cross-correlation
equals the linear valid cross-correlation: out[n,f,i,j] =
sum_{c,p,q} x[n,c,i+p,j+q] * w[f,c,p,q] + bias[f].  That is an ordinary
stride-1 valid conv2d, which maps onto the PE array as 9 accumulated matmuls
(one per filter tap) with C=128 on the contraction partitions, float32r
operands (full-rate fp32 path, ~1.3e-4 rel err), fp32 PSUM accumulation.

Sharding: data-parallel over N (64 samples -> 8 per core), filter replicated.

Raw bass (no Tile scheduler): 5 semaphores, hand-placed waits.  Per core:
  Sync    engine: 16 x-input DMA triggers (sample halves, double-buffered x3)
  Scalar  engine: bias + 9 w-tap DMAs, then per chunk ACTIVATE(+bias) + out DMA
  Tensor  engine: 16 chunks x 9 accumulated matmuls [128c x 128f x 450px]
"""

import numpy as np

import concourse.bass as bass
import concourse.bacc as bacc
import concourse.mybir as mybir
from concourse.bass_utils import run_bass_kernel_spmd

dt = mybir.dt
F32 = dt.float32
F32R = dt.float32r
IDENT = mybir.ActivationFunctionType.Identity

N, C, H, W = 64, 128, 32, 32
F, KH, KW = 128, 3, 3
KK = KH * KW
OH, OW = H - KH + 1, W - KW + 1          # 30, 30
NCORES = 8
NPC = N // NCORES                        # samples per core
RPC = 15                                 # rows per chunk -> 450 px per matmul
NCHUNK = OH // RPC
CPX = RPC * OW                           # 450 <= 512 (one PSUM bank)
NC_CHUNKS = NPC * NCHUNK                 # 16 chunks per core
XBUF, OBUF, PSBUF = 3, 4, 4


def _build():
    nc = bacc.Bacc("TRN2", target_bir_lowering=False, debug=False)

    x_d = nc.dram_tensor("x", [C, NPC, H, W], F32R, kind="ExternalInput").ap()
    w_d = nc.dram_tensor("w", [C, KK, F], F32R, kind="ExternalInput").ap()
    b_d = nc.dram_tensor("bias", [F, 1], F32, kind="ExternalInput").ap()
    o_d = nc.dram_tensor("out", [NPC, F, OH * OW], F32, kind="ExternalOutput").ap()

    w_sb = nc.alloc_sbuf_tensor("w_sb", [C, KK, F], F32R).ap()
    b_sb = nc.alloc_sbuf_tensor("b_sb", [F, 1], F32).ap()
    x_sb = [nc.alloc_sbuf_tensor(f"x_sb{i}", [C, H, W], F32R).ap()
            for i in range(XBUF)]
    o_sb = [nc.alloc_sbuf_tensor(f"o_sb{i}", [F, CPX], F32).ap()
            for i in range(OBUF)]
    ps = [nc.alloc_psum_tensor(f"ps{i}", [F, CPX], F32).ap()
          for i in range(PSBUF)]

    # HWDGE semantics: a DMA's +16 arrives as 16 independent +1s (one per SDMA
    # engine), so a wait at an intermediate threshold on a sem with a second
    # DMA in flight can pass on mixed partial completions.  Sound pattern:
    # dedicate a sem per buffer slot and only ever wait for the maximum value
    # possible at that point (all DMAs issued on that sem so far complete).
    from contextlib import ExitStack
    with ExitStack() as ctx:
      sem = lambda nm: ctx.enter_context(nc.semaphore(nm))
      s_xa = [sem(f"s_xa{j}") for j in range(XBUF)]   # x half A per slot
      s_xb = [sem(f"s_xb{j}") for j in range(XBUF)]   # x half B per slot
      s_wg = [sem(f"s_wg{g}") for g in range(3)]      # w tap groups of 3
      s_b = sem("s_b")
      s_o = [sem(f"s_o{j}") for j in range(OBUF)]     # out DMA per o_sb slot
      s_mm = sem("s_mm")
      s_act = sem("s_act")
      sems = s_xa + s_xb + s_wg + [s_b] + s_o + [s_mm, s_act]

      with nc.Block(no_gpsimd_drain=True) as block:

        @block.sync
        def _(sync):
            for n in range(NPC):
                if n >= XBUF:
                    # slot free once both chunks of sample n-XBUF retired
                    sync.wait_ge(s_mm, 2 * (n - XBUF) + 2)
                sync.dma_start(x_sb[n % XBUF][:, 0:17],
                               x_d[:, n, 0:17]).then_inc(s_xa[n % XBUF], 16)
                sync.dma_start(x_sb[n % XBUF][:, 17:32],
                               x_d[:, n, 17:32]).then_inc(s_xb[n % XBUF], 16)

        @block.scalar
        def _(scalar):
            scalar.dma_start(b_sb[:], b_d[:]).then_inc(s_b, 16)
            for g in range(3):
                scalar.dma_start(w_sb[:, 3 * g:3 * g + 3],
                                 w_d[:, 3 * g:3 * g + 3]).then_inc(s_wg[g], 16)
            for i in range(NC_CHUNKS):
                n, r = divmod(i, 2)
                if i >= OBUF:
                    # o_sb slot free once its previous out DMA fully drained
                    scalar.wait_ge(s_o[i % OBUF], 16 * (i // OBUF))
                if i == 0:
                    scalar.wait_ge(s_b, 16)           # bias landed
                scalar.wait_ge(s_mm, i + 1)           # chunk accumulated
                nc.scalar.activation(o_sb[i % OBUF][:], ps[i % PSBUF][:],
                                     IDENT, bias=b_sb[:]).then_inc(s_act, 1)
                scalar.dma_start(o_d[n, :, r * CPX:(r + 1) * CPX],
                                 o_sb[i % OBUF][:]).then_inc(s_o[i % OBUF], 16)
            for j in range(OBUF):                     # all outputs in DRAM
                scalar.wait_ge(s_o[j], 16 * (NC_CHUNKS // OBUF))

        @block.tensor
        def _(tensor):
            for i in range(NC_CHUNKS):
                n, r = divmod(i, 2)
                # r=0 needs rows 0..16 (half A); r=1 needs rows 15..31 (A+B)
                tensor.wait_ge(s_xa[n % XBUF], 16 * (n // XBUF + 1))
                if r == 1:
                    tensor.wait_ge(s_xb[n % XBUF], 16 * (n // XBUF + 1))
                if i >= PSBUF:
                    tensor.wait_ge(s_act, i - PSBUF + 1)   # bank drained
                for k in range(KK):
                    p, q = divmod(k, KW)
                    if i == 0 and k % 3 == 0:
                        tensor.wait_ge(s_wg[k // 3], 16)   # tap group landed
                    mm = nc.tensor.matmul(
                        ps[i % PSBUF][:],
                        w_sb[:, k],
                        x_sb[n % XBUF][:, r * RPC + p:r * RPC + p + RPC, q:q + OW],
                        start=(k == 0),
                        stop=(k == KK - 1),
                    )
                    if k == KK - 1:
                        mm.then_inc(s_mm, 1)

      # After the block's exit barrier: zero the semaphores so a re-execution
      # of the loaded NEFF starts from a clean state (device sem state
      # persists across executions).
      lo = min(s.num for s in sems)
      hi = max(s.num for s in sems)
      assert hi - lo + 1 == len(sems)
      nc.gpsimd.dma_reset(range(lo, hi + 1))
      nc.gpsimd.sem_clear(range(lo, hi + 1))

    nc.compile()
    return nc


_NC = None


def _get_nc():
    global _NC
    if _NC is None:
        _NC = _build()
    return _NC


def _in_maps(x, w, bias):
    w_prep = np.ascontiguousarray(
        w.transpose(1, 2, 3, 0).reshape(C, KK, F).astype(np.float32))
    b_prep = np.ascontiguousarray(bias.astype(np.float32).reshape(F, 1))
    maps = []
    for c in range(NCORES):
        xc = np.ascontiguousarray(
            x[c * NPC:(c + 1) * NPC].transpose(1, 0, 2, 3).astype(np.float32))
        maps.append({"x": xc, "w": w_prep, "bias": b_prep})
    return maps


def run(x, w, bias, trace=False, **spmd_kwargs):
    """Run the SPMD kernel; returns (out [N,F,OH,OW], BassKernelResults)."""
    nc = _get_nc()
    res = run_bass_kernel_spmd(nc, _in_maps(x, w, bias), list(range(NCORES)),
                               trace=trace, **spmd_kwargs)
    parts = [res.results[c]["out"].reshape(NPC, F, OH, OW) for c in range(NCORES)]
    return np.concatenate(parts, axis=0), res


def kernel(x, w, bias):
    out, _ = run(np.asarray(x), np.asarray(w), np.asarray(bias))
    return out


# revision 14
# speedup vs baseline: 1.0651x; 1.0557x over previous
"""Conv2d-via-FFT reference implemented as a direct convolution on TRN2.

The reference pads to FFT size 61 >= 32+3-1, so its circular cross-correlation
equals the linear valid cross-correlation: out[n,f,i,j] =
sum_{c,p,q} x[n,c,i+p,j+q] * w[f,c,p,q] + bias[f].  That is an ordinary
stride-1 valid conv2d, which maps onto the PE array as 9 accumulated matmuls
(one per filter tap) with C=128 on the contraction partitions, float32r
operands (full-rate fp32 path, ~1.3e-4 rel err), fp32 PSUM accumulation.

Sharding: data-parallel over N (64 samples -> 8 per core), filter replicated.

Raw bass (no Tile scheduler): 5 semaphores, hand-placed waits.  Per core:
  Sync    engine: 16 x-input DMA triggers (sample halves, double-buffered x3)
  Scalar  engine: bias + 9 w-tap DMAs, then per chunk ACTIVATE(+bias) + out DMA
  Tensor  engine: 16 chunks x 9 accumulated matmuls [128c x 128f x 450px]
"""

import numpy as np

import concourse.bass as bass
import concourse.bacc as bacc
import concourse.mybir as mybir
from concourse.bass_utils import run_bass_kernel_spmd

dt = mybir.dt
F32 = dt.float32
F32R = dt.float32r
IDENT = mybir.ActivationFunctionType.Identity

N, C, H, W = 64, 128, 32, 32
F, KH, KW = 128, 3, 3
KK = KH * KW
OH, OW = H - KH + 1, W - KW + 1          # 30, 30
NCORES = 8
NPC = N // NCORES                        # samples per core
RPC = 15                                 # rows per chunk -> 450 px per matmul
NCHUNK = OH // RPC
CPX = RPC * OW                           # 450 <= 512 (one PSUM bank)
NC_CHUNKS = NPC * NCHUNK                 # 16 chunks per core
XBUF, OBUF, PSBUF = 3, 4, 4


def _build():
    nc = bacc.Bacc("TRN2", target_bir_lowering=False, debug=False)

    x_d = nc.dram_tensor("x", [C, NPC, H, W], F32R, kind="ExternalInput").ap()
    w_d = nc.dram_tensor("w", [C, KK, F], F32R, kind="ExternalInput").ap()
    b_d = nc.dram_tensor("bias", [F, 1], F32, kind="ExternalInput").ap()
    o_d = nc.dram_tensor("out", [NPC, F, OH * OW], F32, kind="ExternalOutput").ap()

    w_sb = nc.alloc_sbuf_tensor("w_sb", [C, KK, F], F32R).ap()
    b_sb = nc.alloc_sbuf_tensor("b_sb", [F, 1], F32).ap()
    x_sb = [nc.alloc_sbuf_tensor(f"x_sb{i}", [C, H, W], F32R).ap()
            for i in range(XBUF)]
    o_sb = [nc.alloc_sbuf_tensor(f"o_sb{i}", [F, CPX], F32).ap()
            for i in range(OBUF)]
    ps = [nc.alloc_psum_tensor(f"ps{i}", [F, CPX], F32).ap()
          for i in range(PSBUF)]
    ps_warm = nc.alloc_psum_tensor("ps_warm", [F, 512], F32).ap()

    # HWDGE semantics: a DMA's +16 arrives as 16 independent +1s (one per SDMA
    # engine), so a wait at an intermediate threshold on a sem with a second
    # DMA in flight can pass on mixed partial completions.  Sound pattern:
    # dedicate a sem per buffer slot and only ever wait for the maximum value
    # possible at that point (all DMAs issued on that sem so far complete).
    from contextlib import ExitStack
    with ExitStack() as ctx:
      sem = lambda nm: ctx.enter_context(nc.semaphore(nm))
      s_xa = [sem(f"s_xa{j}") for j in range(XBUF)]   # x half A per slot
      s_xb = [sem(f"s_xb{j}") for j in range(XBUF)]   # x half B per slot
      s_wg = [sem(f"s_wg{g}") for g in range(3)]      # w tap groups of 3
      s_b = sem("s_b")
      s_o = [sem(f"s_o{j}") for j in range(OBUF)]     # out DMA per o_sb slot
      s_mm = sem("s_mm")
      s_act = sem("s_act")
      sems = s_xa + s_xb + s_wg + [s_b] + s_o + [s_mm, s_act]

      with nc.Block(no_gpsimd_drain=True) as block:

        @block.sync
        def _(sync):
            # w group 0 ahead of all x traffic: it is the first LDW dependency
            sync.dma_start(w_sb[:, 0:3], w_d[:, 0:3]).then_inc(s_wg[0], 16)
            for n in range(NPC):
                if n >= XBUF:
                    # slot free once both chunks of sample n-XBUF retired
                    sync.wait_ge(s_mm, 2 * (n - XBUF) + 2)
                sync.dma_start(x_sb[n % XBUF][:, 0:17],
                               x_d[:, n, 0:17]).then_inc(s_xa[n % XBUF], 16)
                sync.dma_start(x_sb[n % XBUF][:, 17:32],
                               x_d[:, n, 17:32]).then_inc(s_xb[n % XBUF], 16)

        @block.scalar
        def _(scalar):
            scalar.dma_start(b_sb[:], b_d[:]).then_inc(s_b, 16)
            for g in range(1, 3):
                scalar.dma_start(w_sb[:, 3 * g:3 * g + 3],
                                 w_d[:, 3 * g:3 * g + 3]).then_inc(s_wg[g], 16)
            for i in range(NC_CHUNKS):
                n, r = divmod(i, 2)
                if i >= OBUF:
                    # o_sb slot free once its previous out DMA fully drained
                    scalar.wait_ge(s_o[i % OBUF], 16 * (i // OBUF))
                if i == 0:
                    scalar.wait_ge(s_b, 16)           # bias landed
                scalar.wait_ge(s_mm, i + 1)           # chunk accumulated
                nc.scalar.activation(o_sb[i % OBUF][:], ps[i % PSBUF][:],
                                     IDENT, bias=b_sb[:]).then_inc(s_act, 1)
                scalar.dma_start(o_d[n, :, r * CPX:(r + 1) * CPX],
                                 o_sb[i % OBUF][:]).then_inc(s_o[i % OBUF], 16)
            for j in range(OBUF):                     # all outputs in DRAM
                scalar.wait_ge(s_o[j], 16 * (NC_CHUNKS // OBUF))

        @block.tensor
        def _(tensor):
            # No-dependency warmup matmuls on whatever is in SBUF: keeps the
            # PE busy from kernel entry so the HAM clock gate opens (K=8/8)
            # before the real matmuls start.  Results land in a scratch bank.
            for _ in range(12):
                nc.tensor.matmul(ps_warm[:], w_sb[:, 0], x_sb[0][:, 0:16, :],
                                 start=True, stop=True)
            for i in range(NC_CHUNKS):
                n, r = divmod(i, 2)
                if i >= PSBUF:
                    tensor.wait_ge(s_act, i - PSBUF + 1)   # bank drained
                if i == 0:
                    tensor.wait_ge(s_wg[0], 16)
                for k in range(KK):
                    p, q = divmod(k, KW)
                    mm = nc.tensor.matmul(
                        ps[i % PSBUF][:],
                        w_sb[:, k],
                        x_sb[n % XBUF][:, r * RPC + p:r * RPC + p + RPC, q:q + OW],
                        start=(k == 0),
                        stop=(k == KK - 1),
                    )
                    if k == 0:
                        # r=0 needs rows 0..16 (half A); r=1 rows 15..31 (A+B;
                        # B's completion implies A's: same ring set, FIFO)
                        mm._wait_ge(s_xa[n % XBUF] if r == 0 else s_xb[n % XBUF],
                                    16 * (n // XBUF + 1))
                    if i == 0 and k in (3, 6):
                        mm._wait_ge(s_wg[k // 3], 16)      # tap group landed
                    if k == KK - 1:
                        mm.then_inc(s_mm, 1)

      # After the block's exit barrier: zero the semaphores so a re-execution
      # of the loaded NEFF starts from a clean state (device sem state
      # persists across executions).
      lo = min(s.num for s in sems)
      hi = max(s.num for s in sems)
      assert hi - lo + 1 == len(sems)
      nc.gpsimd.dma_reset(range(lo, hi + 1))
      nc.gpsimd.sem_clear(range(lo, hi + 1))

    nc.compile()
    return nc


_NC = None


def _get_nc():
    global _NC
    if _NC is None:
        _NC = _build()
    return _NC


def _in_maps(x, w, bias):
    w_prep = np.ascontiguousarray(
        w.transpose(1, 2, 3, 0).reshape(C, KK, F).astype(np.float32))
    b_prep = np.ascontiguousarray(bias.astype(np.float32).reshape(F, 1))
    maps = []
    for c in range(NCORES):
        xc = np.ascontiguousarray(
            x[c * NPC:(c + 1) * NPC].transpose(1, 0, 2, 3).astype(np.float32))
        maps.append({"x": xc, "w": w_prep, "bias": b_prep})
    return maps


def run(x, w, bias, trace=False, **spmd_kwargs):
    """Run the SPMD kernel; returns (out [N,F,OH,OW], BassKernelResults)."""
    nc = _get_nc()
    res = run_bass_kernel_spmd(nc, _in_maps(x, w, bias), list(range(NCORES)),
                               trace=trace, **spmd_kwargs)
    parts = [res.results[c]["out"].reshape(NPC, F, OH, OW) for c in range(NCORES)]
    return np.concatenate(parts, axis=0), res


def kernel(x, w, bias):
    out, _ = run(np.asarray(x), np.asarray(w), np.asarray(bias))
    return out


# revision 17
# speedup vs baseline: 1.0856x; 1.0192x over previous
"""Conv2d-via-FFT reference implemented as a direct convolution on TRN2.

The reference pads to FFT size 61 >= 32+3-1, so its circular cross-correlation
equals the linear valid cross-correlation: out[n,f,i,j] =
sum_{c,p,q} x[n,c,i+p,j+q] * w[f,c,p,q] + bias[f].  That is an ordinary
stride-1 valid conv2d, which maps onto the PE array as 9 accumulated matmuls
(one per filter tap) with C=128 on the contraction partitions, float32r
operands (full-rate fp32 path, ~1.3e-4 rel err), fp32 PSUM accumulation.

Sharding: data-parallel over N (64 samples -> 8 per core), filter replicated.

Raw bass (no Tile scheduler): 5 semaphores, hand-placed waits.  Per core:
  Sync    engine: 16 x-input DMA triggers (sample halves, double-buffered x3)
  Scalar  engine: bias + 9 w-tap DMAs, then per chunk ACTIVATE(+bias) + out DMA
  Tensor  engine: 16 chunks x 9 accumulated matmuls [128c x 128f x 450px]
"""

import numpy as np

import concourse.bass as bass
import concourse.bacc as bacc
import concourse.mybir as mybir
from concourse.bass_utils import run_bass_kernel_spmd

dt = mybir.dt
F32 = dt.float32
F32R = dt.float32r
IDENT = mybir.ActivationFunctionType.Identity

N, C, H, W = 64, 128, 32, 32
F, KH, KW = 128, 3, 3
KK = KH * KW
OH, OW = H - KH + 1, W - KW + 1          # 30, 30
NCORES = 8
NPC = N // NCORES                        # samples per core
RPC = 15                                 # rows per chunk -> 450 px per matmul
NCHUNK = OH // RPC
CPX = RPC * OW                           # 450 <= 512 (one PSUM bank)
NC_CHUNKS = NPC * NCHUNK                 # 16 chunks per core
XBUF, OBUF, PSBUF = 3, 4, 4


def _build():
    nc = bacc.Bacc("TRN2", target_bir_lowering=False, debug=False)

    x_d = nc.dram_tensor("x", [C, NPC, H, W], F32R, kind="ExternalInput").ap()
    w_d = nc.dram_tensor("w", [C, KK, F], F32R, kind="ExternalInput").ap()
    b_d = nc.dram_tensor("bias", [F, 1], F32, kind="ExternalInput").ap()
    o_d = nc.dram_tensor("out", [NPC, F, OH * OW], F32, kind="ExternalOutput").ap()

    w_sb = nc.alloc_sbuf_tensor("w_sb", [C, KK, F], F32R).ap()
    b_sb = nc.alloc_sbuf_tensor("b_sb", [F, 1], F32).ap()
    x_sb = [nc.alloc_sbuf_tensor(f"x_sb{i}", [C, H, W], F32R).ap()
            for i in range(XBUF)]
    o_sb = [nc.alloc_sbuf_tensor(f"o_sb{i}", [F, CPX], F32).ap()
            for i in range(OBUF)]
    ps = [nc.alloc_psum_tensor(f"ps{i}", [F, CPX], F32).ap()
          for i in range(PSBUF)]
    ps_warm = nc.alloc_psum_tensor("ps_warm", [F, 512], F32).ap()

    # HWDGE semantics: a DMA's +16 arrives as 16 independent +1s (one per SDMA
    # engine), so a wait at an intermediate threshold on a sem with a second
    # DMA in flight can pass on mixed partial completions.  Sound pattern:
    # dedicate a sem per buffer slot and only ever wait for the maximum value
    # possible at that point (all DMAs issued on that sem so far complete).
    # Sem numbers are pinned into 60..75: the NEFF epilogue blanket-resets all
    # 249 kernel sems split per engine (~50 each, ~115ns/sem), and the Sync
    # engine owns the 207..255 slice (the only slice inside the bass-visible
    # 155..255 pool whose owner we can make finish last).  Sync gates on the
    # all-outputs-landed waits, so its reset of live sems is ordered after
    # completion, no exit barrier is needed, and the other engines' reset
    # storms overlap compute.
    from contextlib import ExitStack
    with ExitStack() as ctx:
      _next_num = iter(range(207, 250))
      sem = lambda nm: ctx.enter_context(nc.semaphore(nm, num=next(_next_num)))
      s_xa = [sem(f"s_xa{j}") for j in range(XBUF)]   # x half A per slot
      s_xb = [sem(f"s_xb{j}") for j in range(XBUF)]   # x half B per slot
      s_wg = [sem(f"s_wg{g}") for g in range(3)]      # w tap groups of 3
      s_b = sem("s_b")
      s_o = [sem(f"s_o{j}") for j in range(OBUF)]     # out DMA per o_sb slot
      s_mm = sem("s_mm")
      s_act = sem("s_act")

      _orig_barrier = nc.all_engine_barrier
      nc.all_engine_barrier = lambda *a, **k: None
      with nc.Block(no_gpsimd_drain=True) as block:

        @block.sync
        def _(sync):
            # w group 0 ahead of all x traffic: it is the first LDW dependency
            sync.dma_start(w_sb[:, 0:3], w_d[:, 0:3]).then_inc(s_wg[0], 16)
            for n in range(NPC):
                if n >= XBUF:
                    # slot free once both chunks of sample n-XBUF retired
                    sync.wait_ge(s_mm, 2 * (n - XBUF) + 2)
                sync.dma_start(x_sb[n % XBUF][:, 0:17],
                               x_d[:, n, 0:17]).then_inc(s_xa[n % XBUF], 16)
                sync.dma_start(x_sb[n % XBUF][:, 17:32],
                               x_d[:, n, 17:32]).then_inc(s_xb[n % XBUF], 16)
            for j in range(OBUF):                     # all outputs in DRAM
                sync.wait_ge(s_o[j], 16 * (NC_CHUNKS // OBUF))

        @block.scalar
        def _(scalar):
            scalar.dma_start(b_sb[:], b_d[:]).then_inc(s_b, 16)
            for g in range(1, 3):
                scalar.dma_start(w_sb[:, 3 * g:3 * g + 3],
                                 w_d[:, 3 * g:3 * g + 3]).then_inc(s_wg[g], 16)
            for i in range(NC_CHUNKS):
                n, r = divmod(i, 2)
                if i >= OBUF:
                    # o_sb slot free once its previous out DMA fully drained
                    scalar.wait_ge(s_o[i % OBUF], 16 * (i // OBUF))
                if i == 0:
                    scalar.wait_ge(s_b, 16)           # bias landed
                scalar.wait_ge(s_mm, i + 1)           # chunk accumulated
                nc.scalar.activation(o_sb[i % OBUF][:], ps[i % PSBUF][:],
                                     IDENT, bias=b_sb[:]).then_inc(s_act, 1)
                scalar.dma_start(o_d[n, :, r * CPX:(r + 1) * CPX],
                                 o_sb[i % OBUF][:]).then_inc(s_o[i % OBUF], 16)

        @block.tensor
        def _(tensor):
            # No-dependency warmup matmuls on whatever is in SBUF: keeps the
            # PE busy from kernel entry so the HAM clock gate opens (K=8/8)
            # before the real matmuls start.  Results land in a scratch bank.
            for _ in range(12):
                nc.tensor.matmul(ps_warm[:], w_sb[:, 0], x_sb[0][:, 0:16, :],
                                 start=True, stop=True)
            for i in range(NC_CHUNKS):
                n, r = divmod(i, 2)
                if i >= PSBUF:
                    tensor.wait_ge(s_act, i - PSBUF + 1)   # bank drained
                if i == 0:
                    tensor.wait_ge(s_wg[0], 16)
                for k in range(KK):
                    p, q = divmod(k, KW)
                    mm = nc.tensor.matmul(
                        ps[i % PSBUF][:],
                        w_sb[:, k],
                        x_sb[n % XBUF][:, r * RPC + p:r * RPC + p + RPC, q:q + OW],
                        start=(k == 0),
                        stop=(k == KK - 1),
                    )
                    if k == 0:
                        # r=0 needs rows 0..16 (half A); r=1 rows 15..31 (A+B;
                        # B's completion implies A's: same ring set, FIFO)
                        mm._wait_ge(s_xa[n % XBUF] if r == 0 else s_xb[n % XBUF],
                                    16 * (n // XBUF + 1))
                    if i == 0 and k in (3, 6):
                        mm._wait_ge(s_wg[k // 3], 16)      # tap group landed
                    if k == KK - 1:
                        mm.then_inc(s_mm, 1)

      nc.all_engine_barrier = _orig_barrier
      # No explicit sem clear needed: the NEFF epilogue's blanket per-engine
      # reset zeroes every kernel sem, and all increments have retired by the
      # time the Scalar engine (owner of 54..104) reaches its resets.

    nc.compile()
    return nc


_NC = None


def _get_nc():
    global _NC
    if _NC is None:
        _NC = _build()
    return _NC


def _in_maps(x, w, bias):
    w_prep = np.ascontiguousarray(
        w.transpose(1, 2, 3, 0).reshape(C, KK, F).astype(np.float32))
    b_prep = np.ascontiguousarray(bias.astype(np.float32).reshape(F, 1))
    maps = []
    for c in range(NCORES):
        xc = np.ascontiguousarray(
            x[c * NPC:(c + 1) * NPC].transpose(1, 0, 2, 3).astype(np.float32))
        maps.append({"x": xc, "w": w_prep, "bias": b_prep})
    return maps


def run(x, w, bias, trace=False, **spmd_kwargs):
    """Run the SPMD kernel; returns (out [N,F,OH,OW], BassKernelResults)."""
    nc = _get_nc()
    res = run_bass_kernel_spmd(nc, _in_maps(x, w, bias), list(range(NCORES)),
                               trace=trace, **spmd_kwargs)
    parts = [res.results[c]["out"].reshape(NPC, F, OH, OW) for c in range(NCORES)]
    return np.concatenate(parts, axis=0), res


def kernel(x, w, bias):
    out, _ = run(np.asarray(x), np.asarray(w), np.asarray(bias))
    return out
